# revision 36
# baseline (speedup 1.0000x reference)
"""Causal self-attention (B=8, T=1024, C=768, H=12, D=64) on 8 TRN2 NeuronCores.

Sharding: data-parallel over batch — core b handles batch element b. No
collectives. Host pre-transposes x to x^T[b] and pre-casts operands to bf16;
all matmuls run bf16 with fp32 PSUM accumulation.

Per-core algorithm:
  v = x Wv in [t, c] layout (x^T stationary); v bias folded into the y^T
  stage (exact: softmax rows sum to 1). q^T,k^T = (Wqkv^T x^T + b) in
  [c3, t] layout (weights stationary). Per head h, key-block j (128 keys):
  S^T = K_j Q^T in PSUM [keys, q] (causal: only q >= 128j columns; blocks
  pack into [128,1024] PSUM tiles as {j0},{j1,j7},{j2,j6},{j3,j5},{j4} so
  one ACT exp covers each tile, scale=1/8), triangular mask-multiply on
  diagonal 128x128 blocks. Per q-tile i: y'[q,65] = sum_j P_j^T.T @ [V_j|1]
  accumulated in PSUM (two heads x two i-steps share one PSUM bank); col 64
  is the softmax denominator. Normalize via per-partition reciprocal+scale,
  PE-transpose into a per-pair [128, 1024] bf16 PSUM strip (head parity in
  partition halves), one DVE pass per pair adds the v-bias and lands y^T in
  SBUF. out[t, c] = y^T.T @ Wproj + b_proj (bias via K=1 ones matmul).

Emission is software-pipelined (static per-engine order => head-of-line
blocking): cycle hp interleaves AV(hp) i-steps with qk(hp+1) half-chunks
and S(hp+1) groups so PE fills ACT-paced exp stalls; v tiles fill the S(0)
cold start.

PSUM budget (8 banks): big [128,1024]fp32 x2 (4, shared v/qk/S/o) +
y' [128,512]fp32 x2 (2, two heads x two i-steps packed) +
tr [128,1024]bf16 x2 (2).
"""

import numpy as np
import ml_dtypes

B, T, C = 8, 1024, 768
H, D = 12, 64
C3 = 3 * C
KC = C // 128          # 6 contraction chunks over c_in
TT = T // 128          # 8 t-tiles of 128
NPAIR = H // 2

BIG_BUFS = 3
SM_BUFS = 1
TR_BUFS = 1
PP_BUFS = 20           # 10 P segs live per pair (5 groups x 2 heads)

_BF16 = ml_dtypes.bfloat16

_compiled = {}


def _build():
    from concourse import bacc, mybir
    import concourse.tile as tile
    from concourse.masks import make_identity, make_upper_triangular

    fp32 = mybir.dt.float32
    bf16 = mybir.dt.bfloat16

    nc = bacc.Bacc("TRN2", target_bir_lowering=False, debug=False,
                   enable_asserts=True, num_devices=B)

    xT = nc.dram_tensor("xT", [C, T], bf16, kind="ExternalInput")
    wqkv = nc.dram_tensor("wqkv", [C, C3], bf16, kind="ExternalInput")
    wproj = nc.dram_tensor("wproj", [C, C], bf16, kind="ExternalInput")
    # b_qkv rearranged host-side to [128, 18]: col j holds b_qkv[128j:128j+128]
    bqkv = nc.dram_tensor("bqkv", [128, C3 // 128], fp32, kind="ExternalInput")
    bproj = nc.dram_tensor("bproj", [1, C], bf16, kind="ExternalInput")
    out = nc.dram_tensor("out", [T, C], bf16, kind="ExternalOutput")

    Exp = mybir.ActivationFunctionType.Exp
    # S-block packing: groups of (j, base column) sharing one [128,1024]
    # PSUM tile => one exp per tile. Bases keep each block inside the tile.
    GROUPS = [((4, 0),), ((3, 0), (5, 640)), ((2, 0), (6, 768)),
              ((1, 0), (7, 896)), ((0, 0),)]

    with tile.TileContext(nc) as tc:
        with (
            tc.tile_pool(name="const", bufs=1) as const,
            tc.tile_pool(name="pP", bufs=PP_BUFS) as pP,
            tc.tile_pool(name="small", bufs=6) as small,
            tc.tile_pool(name="osb", bufs=4) as osb,
            tc.tile_pool(name="ps_big", bufs=BIG_BUFS, space="PSUM") as ps_big,
            tc.tile_pool(name="ps_sm", bufs=SM_BUFS, space="PSUM") as ps_sm,
            tc.tile_pool(name="ps_tr", bufs=TR_BUFS, space="PSUM") as ps_tr,
        ):
            # ---- persistent SBUF loads ----
            # Merged [128, KC, *] tiles allow few, large, >=512B-row DMAs.
            # Critical path: per-chunk q/k pair-0/1 slivers (SP queue)
            # interleaved with per-chunk full x^T (ACT queue) feed a
            # chunk-major qk(0); everything else streams behind them.
            xT_all = const.tile([128, KC, T], bf16, tag="xT", name="xT_all")
            wqkv_all = const.tile([128, KC, C3], bf16, tag="wqkv",
                                  name="wqkv_all")
            wproj_all = const.tile([128, KC, C], bf16, tag="wproj",
                                   name="wproj_all")
            xT_sb = [xT_all[:, kc] for kc in range(KC)]
            wqkv_sb = [wqkv_all[:, kc] for kc in range(KC)]
            wproj_sb = [wproj_all[:, kc] for kc in range(KC)]
            wqkv_pac = wqkv.rearrange("(a p) c -> p a c", p=128)
            bqkv_sb = const.tile([128, C3 // 128], fp32, tag="bqkv", name="bqkv")
            bproj_sb = const.tile([1, C], bf16, tag="bproj", name="bproj")
            nc.gpsimd.dma_start(bqkv_sb[:], bqkv[:, :])
            nc.gpsimd.dma_start(bproj_sb[:], bproj[:, :])
            for half in range(2):
                kcs = slice(3 * half, 3 * half + 3)
                for base in (0, C):
                    nc.sync.dma_start(
                        wqkv_all[:, kcs, base:base + 256],
                        wqkv_pac[:, kcs, base:base + 256],
                    )
            for kc in range(KC):
                nc.scalar.dma_start(xT_all[:, kc],
                                    xT[kc * 128:(kc + 1) * 128, :])
            # Bulk loads ride the same two HWDGE queues BEHIND the critical
            # stream (SWDGE would jump the shared DMA device): v columns +
            # q-rest on SP, k-rest + wproj on ACT.
            nc.sync.dma_start(wqkv_all[:, :, 2 * C:C3],
                              wqkv_pac[:, :, 2 * C:C3])
            nc.sync.dma_start(wqkv_all[:, :, 256:768],
                              wqkv_pac[:, :, 256:768])
            nc.scalar.dma_start(wqkv_all[:, :, C + 256:C + 768],
                                wqkv_pac[:, :, C + 256:C + 768])
            nc.scalar.dma_start(wproj_all[:],
                                wproj.rearrange("(a p) c -> p a c", p=128))
            ones_sb = const.tile([1, 128], bf16, tag="ones", name="ones")
            nc.vector.memset(ones_sb[:], 1.0)
            # b_proj broadcast to all 128 partitions (PE trick, done once in
            # the prologue); proj adds it via DVE instead of per-tile matmuls
            bias_sb = const.tile([128, C], bf16, tag="bias", name="bias")
            ident_sb = const.tile([128, 128], bf16, tag="ident", name="ident")
            make_identity(nc, ident_sb[:])
            # keep columns m >= l (query >= key) on the diagonal block
            trimask_sb = const.tile([128, 128], bf16, tag="trimask", name="trimask")
            make_upper_triangular(nc, trimask_sb[:], val=1.0, diag=True)

            qkT_sb = [const.tile([128, T], bf16, tag=f"qkT{c3}", name=f"qkT{c3}")
                      for c3 in range(2 * KC)]
            # v packed [t, 12 heads x (64 + ones col)]
            v_sb = [const.tile([128, H, D + 1], bf16, tag=f"v{tt}", name=f"v{tt}")
                    for tt in range(TT)]
            yT_sb = [const.tile([128, T], bf16, tag=f"yT{kc}", name=f"yT{kc}")
                     for kc in range(KC)]

            def emit_v(tt):
                ps = ps_big.tile([128, 1024], fp32, tag="big", name="v_ps")
                for kc in range(KC):
                    nc.tensor.matmul(
                        ps[:, 0:512],
                        xT_sb[kc][:, tt * 128:(tt + 1) * 128],
                        wqkv_sb[kc][:, 2 * C:2 * C + 512],
                        start=(kc == 0), stop=(kc == KC - 1),
                    )
                    nc.tensor.matmul(
                        ps[:, 512:768],
                        xT_sb[kc][:, tt * 128:(tt + 1) * 128],
                        wqkv_sb[kc][:, 2 * C + 512:3 * C],
                        start=(kc == 0), stop=(kc == KC - 1),
                    )
                vv = v_sb[tt]
                nc.vector.tensor_copy(
                    vv[:, :, 0:D],
                    ps[:, 0:768].rearrange("p (h d) -> p h d", d=D),
                )
                nc.vector.memset(vv[:, :, D:D + 1], 1.0)

            def emit_qk_half(hp, which, tchunk):
                c3 = hp if which == "q" else KC + hp
                ps = ps_big.tile([128, 1024], fp32, tag="big", name="qk_ps")
                sl = slice(tchunk * 512, (tchunk + 1) * 512)
                for kc in range(KC):
                    nc.tensor.matmul(
                        ps[:, sl],
                        wqkv_sb[kc][:, c3 * 128:(c3 + 1) * 128],
                        xT_sb[kc][:, sl],
                        start=(kc == 0), stop=(kc == KC - 1),
                    )
                nc.vector.tensor_scalar_add(
                    qkT_sb[c3][:, sl], ps[:, sl], bqkv_sb[:, c3:c3 + 1],
                )

            def emit_S_group(hp, segs, grp):
                qT = qkT_sb[hp]
                kT = qkT_sb[KC + hp]
                for h in (2 * hp, 2 * hp + 1):
                    poff = 64 * (h % 2)
                    S = ps_big.tile([128, 1024], fp32, tag="big", name="S")
                    span = 0
                    for j, base in grp:
                        qs = 128 * j
                        w = T - qs
                        span = base + w
                        first = base + min(512 - base % 512, w) if base < 512 \
                            else base + w
                        for a, b_ in ((base, first), (first, base + w)):
                            if b_ <= a:
                                continue
                            nc.tensor.matmul(
                                S[:, a:b_],
                                kT[poff:poff + 64, qs:qs + 128],
                                qT[poff:poff + 64, qs + (a - base):qs + (b_ - base)],
                                start=True, stop=True,
                            )
                    P = pP.tile([128, 1024], bf16, tag="P", name="P")
                    nc.scalar.activation(P[:, 0:span], S[:, 0:span], Exp,
                                         scale=0.125)
                    for j, base in grp:
                        # diagonal-block mask on GPSIMD: keeps the DVE FIFO
                        # off the AV->normalize latency loop
                        nc.gpsimd.tensor_mul(P[:, base:base + 128],
                                             P[:, base:base + 128],
                                             trimask_sb[:])
                        segs[h][j] = (P, base)

            def emit_S_j4_pair(hp, segs):
                # both heads' j4 block (512 cols each) share one PSUM tile
                # and one exp: halves the ACT op overhead for this group
                qT = qkT_sb[hp]
                kT = qkT_sb[KC + hp]
                S = ps_big.tile([128, 1024], fp32, tag="big", name="S")
                for idx, h in enumerate((2 * hp, 2 * hp + 1)):
                    poff = 64 * (h % 2)
                    nc.tensor.matmul(
                        S[:, 512 * idx:512 * idx + 512],
                        kT[poff:poff + 64, 512:640],
                        qT[poff:poff + 64, 512:1024],
                        start=True, stop=True,
                    )
                P = pP.tile([128, 1024], bf16, tag="P", name="P")
                nc.scalar.activation(P[:], S[:], Exp, scale=0.125)
                for idx, h in enumerate((2 * hp, 2 * hp + 1)):
                    base = 512 * idx
                    nc.gpsimd.tensor_mul(P[:, base:base + 128],
                                         P[:, base:base + 128],
                                         trimask_sb[:])
                    segs[h][4] = (P, base)

            def emit_AV_half(hp, segs, yns, i, y2, slot):
                # y2 is one persistent [128, 512] PSUM tile; i-steps rotate
                # through three 170-col slots (2 heads x 65 used). Subtile dep
                # tracking gives each new accumulation 3 steps of slack over
                # the previous slot's readers; PE program order makes the
                # in-bank has_written clears safe.
                pair = (2 * hp, 2 * hp + 1)
                b0 = 170 * slot
                for idx, h in enumerate(pair):
                    c0 = b0 + 65 * idx
                    for j in range(i + 1):
                        P, base = segs[h][j]
                        off = base + 128 * (i - j)
                        nc.tensor.matmul(
                            y2[:, c0:c0 + D + 1],
                            P[:, off:off + 128],
                            v_sb[j][:, h, :],
                            start=(j == 0), stop=(j == i),
                        )
                recip = small.tile([128, 2], fp32, tag="recip", name="recip")
                nc.vector.reciprocal(
                    recip[:],
                    y2[:, b0:b0 + 130].rearrange(
                        "p (g c) -> p g c", c=65)[:, :, D],
                )
                # both heads of the pair share one [128, 128] tile so a single
                # PE transpose handles the pair (head parity = column halves).
                # The last cycle normalizes on ACT (idle there) to keep the
                # DVE FIFO off the AV->normalize->AV latency loop.
                yn = small.tile([128, 2 * D], bf16, tag="yn", name="yn",
                                bufs=18)
                Copy = mybir.ActivationFunctionType.Copy
                for idx, h in enumerate(pair):
                    c0 = b0 + 65 * idx
                    if hp == NPAIR - 1:
                        nc.scalar.activation(yn[:, D * idx:D * idx + D],
                                             y2[:, c0:c0 + D], Copy,
                                             scale=recip[:, idx:idx + 1])
                    else:
                        nc.vector.tensor_scalar_mul(yn[:, D * idx:D * idx + D],
                                                    y2[:, c0:c0 + D],
                                                    recip[:, idx:idx + 1])
                yns.append((i, yn))

            def emit_yT(hp, trs):
                nc.vector.tensor_scalar_add(
                    yT_sb[hp][:],
                    trs[:],
                    bqkv_sb[:, 2 * KC + hp:2 * KC + hp + 1],
                )

            def emit_yT_slice(hp, trs, i):
                # per-step slice add (last cycle): lets proj kc=5 chunks for
                # tile i start as soon as transpose(i) lands
                sl = slice(128 * i, 128 * (i + 1))
                nc.vector.tensor_scalar_add(
                    yT_sb[hp][:, sl],
                    trs[:, sl],
                    bqkv_sb[:, 2 * KC + hp:2 * KC + hp + 1],
                )

            def new_segs():
                return {h: {} for h in range(H)}

            # ---- cold start: chunk-major qk(0) — all four halves advance
            # one contraction chunk at a time so PE tracks DMA arrivals ----
            segs = {0: new_segs()}
            qkA = ps_big.tile([128, 1024], fp32, tag="big", name="qkA")
            qkB = ps_big.tile([128, 1024], fp32, tag="big", name="qkB")
            for kc in range(KC):
                f, l = (kc == 0), (kc == KC - 1)
                wq = wqkv_sb[kc][:, 0:128]
                wk = wqkv_sb[kc][:, C:C + 128]
                for ps_, sl in ((qkA, slice(0, 512)), (qkB, slice(512, 1024))):
                    nc.tensor.matmul(ps_[:, 0:512], wq, xT_sb[kc][:, sl],
                                     start=f, stop=l)
                    nc.tensor.matmul(ps_[:, 512:1024], wk, xT_sb[kc][:, sl],
                                     start=f, stop=l)
            # add order matches S(0)-j0 consumption: k-t0 first (its 0:128
            # sliver gates the first S Ldweights), then q halves, k-t1 last
            for ps_, col, sl in (
                (qkA, KC, slice(0, 512)), (qkA, 0, slice(0, 512)),
                (qkB, 0, slice(512, 1024)), (qkB, KC, slice(512, 1024)),
            ):
                src = ps_[:, 0:512] if col == 0 else ps_[:, 512:1024]
                nc.vector.tensor_scalar_add(
                    qkT_sb[col][:, sl], src, bqkv_sb[:, col:col + 1])
            bps = ps_sm.tile([128, 512], fp32, tag="sm", name="bias_ps")
            nc.tensor.matmul(bps[:, 0:512], ones_sb[:], bproj_sb[:, 0:512],
                             start=True, stop=True)
            nc.vector.tensor_copy(bias_sb[:, 0:512], bps[:, 0:512])
            bps2 = ps_sm.tile([128, 512], fp32, tag="sm", name="bias_ps2")
            nc.tensor.matmul(bps2[:, 0:256], ones_sb[:], bproj_sb[:, 512:768],
                             start=True, stop=True)
            nc.vector.tensor_copy(bias_sb[:, 512:768], bps2[:, 0:256])
            for g in range(4):
                emit_S_group(0, segs[0], GROUPS[4 - g])
                emit_v(g)
            emit_S_j4_pair(0, segs[0])
            emit_v(4)
            emit_v(5)

            # ---- pipelined cycles ----
            # per cycle: 8 AV i-steps; qk(hp+1) halves at steps 0,1,3; S(hp+1)
            # groups j0-first at steps 2,4,5,6,7 (j0 consumed first next cycle).
            # Transposes are lag-0: yn(i-1) transposed at step i of the SAME
            # cycle, so yT(hp) lands at cycle end. The last cycle has no
            # qk/S(hp+1) work, so proj chunks kc<=4 for tiles 0-2 pre-fill
            # its PE stalls (big pool is otherwise unused there).
            qk_sched = {0: [("q", 0), ("k", 0)], 1: [("q", 1)], 3: [("k", 1)]}
            def emit_transpose_slice(trs, yns, sl):
                for i, yn in yns[sl]:
                    nc.tensor.transpose(
                        trs[:, 128 * i:128 * (i + 1)],
                        yn[:], ident_sb[:])

            def emit_proj_chunk(ps, tt, kc, start, stop):
                for a, b_ in ((0, 512), (512, 768)):
                    nc.tensor.matmul(
                        ps[:, a:b_],
                        yT_sb[kc][:, tt * 128:(tt + 1) * 128],
                        wproj_sb[kc][:, a:b_],
                        start=start, stop=stop,
                    )

            # Last-cycle proj schedule: tiles 2-7 fully accumulate (kc 0..5)
            # inside cycle 5 — tile tt's kc=5 chunk only needs the 128-col
            # slice of yT(5) produced by transpose(tt), which lands per-step
            # (sliced yT adds in the last cycle). Out-adds free big bufs so
            # three tiles are in flight at a time; tiles 0-1 run post-loop,
            # overlapping the drain.
            proj_ps = {}
            # step -> list of ("chunks", tt, kcs) | ("out", tt)
            c5_sched = {
                0: [("chunks", 2, (0, 1, 2))],
                1: [("chunks", 2, (3, 4)), ("chunks", 3, (0,))],
                2: [("chunks", 3, (1, 2, 3, 4))],
                3: [("chunks", 4, (0, 1, 2))],
                4: [("chunks", 4, (3, 4)), ("chunks", 2, (5,)), ("out", 2)],
                5: [("chunks", 5, (0, 1, 2)), ("chunks", 3, (5,)), ("out", 3)],
                6: [("chunks", 5, (3, 4)), ("chunks", 6, (0, 1, 2)),
                    ("chunks", 4, (5,)), ("out", 4)],
                7: [("chunks", 6, (3, 4)), ("chunks", 7, (0, 1, 2, 3, 4)),
                    ("chunks", 5, (5,)), ("out", 5)],
            }

            def emit_proj_out(tt, ps, piece=False):
                o = osb.tile([128, C], bf16, tag="o_sb", name="o_sb")
                q = nc.scalar if tt % 2 else nc.sync
                if not piece:
                    nc.vector.tensor_add(o[:], ps[:, 0:768], bias_sb[:])
                    q.dma_start(out[tt * 128:(tt + 1) * 128, :], o[:])
                else:
                    for a, b_ in ((0, 384), (384, 640), (640, 768)):
                        nc.vector.tensor_add(o[:, a:b_], ps[:, a:b_],
                                             bias_sb[:, a:b_])
                        q.dma_start(
                            out[tt * 128:(tt + 1) * 128, a:b_], o[:, a:b_])

            def emit_c5(action):
                kind, tt = action[0], action[1]
                if kind == "chunks":
                    if tt not in proj_ps:
                        proj_ps[tt] = ps_big.tile([128, 1024], fp32,
                                                  tag="big", name="o_ps")
                    for kc in action[2]:
                        emit_proj_chunk(proj_ps[tt], tt, kc,
                                        start=(kc == 0), stop=(kc == 5))
                else:
                    emit_proj_out(tt, proj_ps.pop(tt))

            y2_tile = ps_sm.tile([128, 512], fp32, tag="sm", name="y2")
            for hp in range(NPAIR):
                nxt = hp + 1 < NPAIR
                if nxt:
                    segs[hp + 1] = new_segs()
                yns = []
                trs = ps_tr.tile([128, 1024], bf16, tag="tr", name="tr")
                for i in range(TT):
                    emit_AV_half(hp, segs[hp], yns, i, y2_tile, (hp * TT + i) % 3)
                    if hp == 0 and i in (3, 5):
                        emit_v(6 if i == 3 else 7)
                    if i > 0:
                        emit_transpose_slice(trs, yns, slice(i - 1, i))
                        if not nxt:
                            emit_yT_slice(hp, trs, i - 1)
                    if nxt:
                        for args in qk_sched.get(i, []):
                            emit_qk_half(hp + 1, *args)
                        gidx = {2: 4, 4: 3, 5: 2, 6: 1}.get(i)
                        if gidx is not None:
                            emit_S_group(hp + 1, segs[hp + 1], GROUPS[gidx])
                        elif i == 7:
                            emit_S_j4_pair(hp + 1, segs[hp + 1])
                    else:
                        for action in c5_sched.get(i, []):
                            emit_c5(action)
                emit_transpose_slice(trs, yns, slice(TT - 1, TT))
                if nxt:
                    emit_yT(hp, trs)
                else:
                    emit_yT_slice(hp, trs, TT - 1)
                segs.pop(hp)

            # ---- drain: remaining kc5 chunks, then tiles 0-1 in full ----
            for action in (("chunks", 6, (5,)), ("out", 6),
                           ("chunks", 7, (5,)), ("out", 7)):
                emit_c5(action)
            for tt in (0, 1):
                ps = ps_big.tile([128, 1024], fp32, tag="big", name="o_ps")
                for kc in range(KC):
                    emit_proj_chunk(ps, tt, kc, start=(kc == 0),
                                    stop=(kc == KC - 1))
                emit_proj_out(tt, ps, piece=(tt == 1))

    nc.compile()
    return nc


def _prep_inputs(x, w_qkv, b_qkv, w_proj, b_proj):
    wqkv_bf = np.ascontiguousarray(w_qkv.astype(_BF16))
    wproj_bf = np.ascontiguousarray(w_proj.astype(_BF16))
    bqkv_pc = np.ascontiguousarray(b_qkv.astype(np.float32).reshape(C3 // 128, 128).T)
    bproj_bf = np.ascontiguousarray(b_proj.astype(_BF16).reshape(1, C))
    in_maps = []
    for b in range(B):
        xTb = np.ascontiguousarray(x[b].astype(_BF16).T)
        in_maps.append({
            "xT": xTb,
            "wqkv": wqkv_bf,
            "wproj": wproj_bf,
            "bqkv": bqkv_pc,
            "bproj": bproj_bf,
        })
    return in_maps


def _run(inputs, trace=False):
    from concourse.bass_utils import run_bass_kernel_spmd

    if "nc" not in _compiled:
        _compiled["nc"] = _build()
    nc = _compiled["nc"]
    in_maps = _prep_inputs(inputs["x"], inputs["w_qkv"], inputs["b_qkv"],
                           inputs["w_proj"], inputs["b_proj"])
    res = run_bass_kernel_spmd(nc, in_maps, list(range(B)), trace=trace)
    outs = np.stack([np.asarray(res.results[b]["out"]) for b in range(B)])
    return outs.astype(np.float32), res


def kernel(x, w_qkv, b_qkv, w_proj, b_proj):
    outs, _ = _run(dict(x=x, w_qkv=w_qkv, b_qkv=b_qkv,
                        w_proj=w_proj, b_proj=b_proj))
    return outs



# revision 37
# speedup vs baseline: 1.0038x; 1.0038x over previous
"""Causal self-attention (B=8, T=1024, C=768, H=12, D=64) on 8 TRN2 NeuronCores.

Sharding: data-parallel over batch — core b handles batch element b. No
collectives. Host pre-transposes x to x^T[b] and pre-casts operands to bf16;
all matmuls run bf16 with fp32 PSUM accumulation.

Per-core algorithm:
  v = x Wv in [t, c] layout (x^T stationary); v bias folded into the y^T
  stage (exact: softmax rows sum to 1). q^T,k^T = (Wqkv^T x^T + b) in
  [c3, t] layout (weights stationary). Per head h, key-block j (128 keys):
  S^T = K_j Q^T in PSUM [keys, q] (causal: only q >= 128j columns; blocks
  pack into [128,1024] PSUM tiles as {j0},{j1,j7},{j2,j6},{j3,j5},{j4} so
  one ACT exp covers each tile, scale=1/8), triangular mask-multiply on
  diagonal 128x128 blocks. Per q-tile i: y'[q,65] = sum_j P_j^T.T @ [V_j|1]
  accumulated in PSUM (two heads x two i-steps share one PSUM bank); col 64
  is the softmax denominator. Normalize via per-partition reciprocal+scale,
  PE-transpose into a per-pair [128, 1024] bf16 PSUM strip (head parity in
  partition halves), one DVE pass per pair adds the v-bias and lands y^T in
  SBUF. out[t, c] = y^T.T @ Wproj + b_proj (bias via K=1 ones matmul).

Emission is software-pipelined (static per-engine order => head-of-line
blocking): cycle hp interleaves AV(hp) i-steps with qk(hp+1) half-chunks
and S(hp+1) groups so PE fills ACT-paced exp stalls; v tiles fill the S(0)
cold start.

PSUM budget (8 banks): big [128,1024]fp32 x2 (4, shared v/qk/S/o) +
y' [128,512]fp32 x2 (2, two heads x two i-steps packed) +
tr [128,1024]bf16 x2 (2).
"""

import numpy as np
import ml_dtypes

B, T, C = 8, 1024, 768
H, D = 12, 64
C3 = 3 * C
KC = C // 128          # 6 contraction chunks over c_in
TT = T // 128          # 8 t-tiles of 128
NPAIR = H // 2

BIG_BUFS = 3
SM_BUFS = 1
TR_BUFS = 1
PP_BUFS = 20           # 10 P segs live per pair (5 groups x 2 heads)

_BF16 = ml_dtypes.bfloat16

_compiled = {}


def _build():
    from concourse import bacc, mybir
    import concourse.tile as tile
    from concourse.masks import make_identity, make_upper_triangular

    fp32 = mybir.dt.float32
    bf16 = mybir.dt.bfloat16

    nc = bacc.Bacc("TRN2", target_bir_lowering=False, debug=False,
                   enable_asserts=True, num_devices=B)

    xT = nc.dram_tensor("xT", [C, T], bf16, kind="ExternalInput")
    wqkv = nc.dram_tensor("wqkv", [C, C3], bf16, kind="ExternalInput")
    wproj = nc.dram_tensor("wproj", [C, C], bf16, kind="ExternalInput")
    # b_qkv rearranged host-side to [128, 18]: col j holds b_qkv[128j:128j+128]
    bqkv = nc.dram_tensor("bqkv", [128, C3 // 128], fp32, kind="ExternalInput")
    bproj = nc.dram_tensor("bproj", [1, C], bf16, kind="ExternalInput")
    out = nc.dram_tensor("out", [T, C], bf16, kind="ExternalOutput")

    Exp = mybir.ActivationFunctionType.Exp
    # S-block packing: groups of (j, base column) sharing one [128,1024]
    # PSUM tile => one exp per tile. Bases keep each block inside the tile.
    GROUPS = [((4, 0),), ((3, 0), (5, 640)), ((2, 0), (6, 768)),
              ((1, 0), (7, 896)), ((0, 0),)]

    with tile.TileContext(nc) as tc:
        with (
            tc.tile_pool(name="const", bufs=1) as const,
            tc.tile_pool(name="pP", bufs=PP_BUFS) as pP,
            tc.tile_pool(name="small", bufs=6) as small,
            tc.tile_pool(name="osb", bufs=4) as osb,
            tc.tile_pool(name="ps_big", bufs=BIG_BUFS, space="PSUM") as ps_big,
            tc.tile_pool(name="ps_sm", bufs=SM_BUFS, space="PSUM") as ps_sm,
            tc.tile_pool(name="ps_tr", bufs=TR_BUFS, space="PSUM") as ps_tr,
        ):
            # ---- persistent SBUF loads ----
            # Merged [128, KC, *] tiles allow few, large, >=512B-row DMAs.
            # Critical path: per-chunk q/k pair-0/1 slivers (SP queue)
            # interleaved with per-chunk full x^T (ACT queue) feed a
            # chunk-major qk(0); everything else streams behind them.
            xT_all = const.tile([128, KC, T], bf16, tag="xT", name="xT_all")
            wqkv_all = const.tile([128, KC, C3], bf16, tag="wqkv",
                                  name="wqkv_all")
            wproj_all = const.tile([128, KC, C], bf16, tag="wproj",
                                   name="wproj_all")
            xT_sb = [xT_all[:, kc] for kc in range(KC)]
            wqkv_sb = [wqkv_all[:, kc] for kc in range(KC)]
            wproj_sb = [wproj_all[:, kc] for kc in range(KC)]
            wqkv_pac = wqkv.rearrange("(a p) c -> p a c", p=128)
            bqkv_sb = const.tile([128, C3 // 128], fp32, tag="bqkv", name="bqkv")
            bproj_sb = const.tile([1, C], bf16, tag="bproj", name="bproj")
            nc.gpsimd.dma_start(bqkv_sb[:], bqkv[:, :])
            nc.gpsimd.dma_start(bproj_sb[:], bproj[:, :])
            for half in range(2):
                kcs = slice(3 * half, 3 * half + 3)
                for base in (0, C):
                    nc.sync.dma_start(
                        wqkv_all[:, kcs, base:base + 256],
                        wqkv_pac[:, kcs, base:base + 256],
                    )
            for kc in range(KC):
                nc.scalar.dma_start(xT_all[:, kc],
                                    xT[kc * 128:(kc + 1) * 128, :])
            # Bulk loads ride the same two HWDGE queues BEHIND the critical
            # stream (SWDGE would jump the shared DMA device): v columns +
            # q-rest on SP, k-rest + wproj on ACT.
            nc.sync.dma_start(wqkv_all[:, :, 2 * C:C3],
                              wqkv_pac[:, :, 2 * C:C3])
            nc.sync.dma_start(wqkv_all[:, :, 256:768],
                              wqkv_pac[:, :, 256:768])
            nc.scalar.dma_start(wqkv_all[:, :, C + 256:C + 768],
                                wqkv_pac[:, :, C + 256:C + 768])
            nc.scalar.dma_start(wproj_all[:],
                                wproj.rearrange("(a p) c -> p a c", p=128))
            ones_sb = const.tile([1, 128], bf16, tag="ones", name="ones")
            nc.vector.memset(ones_sb[:], 1.0)
            # b_proj broadcast to all 128 partitions (PE trick, done once in
            # the prologue); proj adds it via DVE instead of per-tile matmuls
            bias_sb = const.tile([128, C], bf16, tag="bias", name="bias")
            ident_sb = const.tile([128, 128], bf16, tag="ident", name="ident")
            make_identity(nc, ident_sb[:])
            # keep columns m >= l (query >= key) on the diagonal block
            trimask_sb = const.tile([128, 128], bf16, tag="trimask", name="trimask")
            make_upper_triangular(nc, trimask_sb[:], val=1.0, diag=True)

            qkT_sb = [const.tile([128, T], bf16, tag=f"qkT{c3}", name=f"qkT{c3}")
                      for c3 in range(2 * KC)]
            # v packed [t, 12 heads x (64 + ones col)]
            v_sb = [const.tile([128, H, D + 1], bf16, tag=f"v{tt}", name=f"v{tt}")
                    for tt in range(TT)]
            yT_sb = [const.tile([128, T], bf16, tag=f"yT{kc}", name=f"yT{kc}")
                     for kc in range(KC)]

            def emit_v(tt):
                ps = ps_big.tile([128, 1024], fp32, tag="big", name="v_ps")
                for kc in range(KC):
                    nc.tensor.matmul(
                        ps[:, 0:512],
                        xT_sb[kc][:, tt * 128:(tt + 1) * 128],
                        wqkv_sb[kc][:, 2 * C:2 * C + 512],
                        start=(kc == 0), stop=(kc == KC - 1),
                    )
                    nc.tensor.matmul(
                        ps[:, 512:768],
                        xT_sb[kc][:, tt * 128:(tt + 1) * 128],
                        wqkv_sb[kc][:, 2 * C + 512:3 * C],
                        start=(kc == 0), stop=(kc == KC - 1),
                    )
                vv = v_sb[tt]
                nc.vector.tensor_copy(
                    vv[:, :, 0:D],
                    ps[:, 0:768].rearrange("p (h d) -> p h d", d=D),
                )
                nc.vector.memset(vv[:, :, D:D + 1], 1.0)

            def emit_qk_half(hp, which, tchunk):
                c3 = hp if which == "q" else KC + hp
                ps = ps_big.tile([128, 1024], fp32, tag="big", name="qk_ps")
                sl = slice(tchunk * 512, (tchunk + 1) * 512)
                for kc in range(KC):
                    nc.tensor.matmul(
                        ps[:, sl],
                        wqkv_sb[kc][:, c3 * 128:(c3 + 1) * 128],
                        xT_sb[kc][:, sl],
                        start=(kc == 0), stop=(kc == KC - 1),
                    )
                nc.vector.tensor_scalar_add(
                    qkT_sb[c3][:, sl], ps[:, sl], bqkv_sb[:, c3:c3 + 1],
                )

            def emit_S_group(hp, segs, grp):
                qT = qkT_sb[hp]
                kT = qkT_sb[KC + hp]
                for h in (2 * hp, 2 * hp + 1):
                    poff = 64 * (h % 2)
                    S = ps_big.tile([128, 1024], fp32, tag="big", name="S")
                    span = 0
                    for j, base in grp:
                        qs = 128 * j
                        w = T - qs
                        span = base + w
                        first = base + min(512 - base % 512, w) if base < 512 \
                            else base + w
                        for a, b_ in ((base, first), (first, base + w)):
                            if b_ <= a:
                                continue
                            nc.tensor.matmul(
                                S[:, a:b_],
                                kT[poff:poff + 64, qs:qs + 128],
                                qT[poff:poff + 64, qs + (a - base):qs + (b_ - base)],
                                start=True, stop=True,
                            )
                    P = pP.tile([128, 1024], bf16, tag="P", name="P")
                    nc.scalar.activation(P[:, 0:span], S[:, 0:span], Exp,
                                         scale=0.125)
                    for j, base in grp:
                        nc.vector.tensor_mul(P[:, base:base + 128],
                                             P[:, base:base + 128],
                                             trimask_sb[:])
                        segs[h][j] = (P, base)

            def emit_S_j4_pair(hp, segs):
                # both heads' j4 block (512 cols each) share one PSUM tile
                # and one exp: halves the ACT op overhead for this group
                qT = qkT_sb[hp]
                kT = qkT_sb[KC + hp]
                S = ps_big.tile([128, 1024], fp32, tag="big", name="S")
                for idx, h in enumerate((2 * hp, 2 * hp + 1)):
                    poff = 64 * (h % 2)
                    nc.tensor.matmul(
                        S[:, 512 * idx:512 * idx + 512],
                        kT[poff:poff + 64, 512:640],
                        qT[poff:poff + 64, 512:1024],
                        start=True, stop=True,
                    )
                P = pP.tile([128, 1024], bf16, tag="P", name="P")
                nc.scalar.activation(P[:], S[:], Exp, scale=0.125)
                for idx, h in enumerate((2 * hp, 2 * hp + 1)):
                    base = 512 * idx
                    nc.vector.tensor_mul(P[:, base:base + 128],
                                         P[:, base:base + 128],
                                         trimask_sb[:])
                    segs[h][4] = (P, base)

            def emit_AV_half(hp, segs, yns, i, y2, slot):
                # y2 is one persistent [128, 512] PSUM tile; i-steps rotate
                # through three 170-col slots (2 heads x 65 used). Subtile dep
                # tracking gives each new accumulation 3 steps of slack over
                # the previous slot's readers; PE program order makes the
                # in-bank has_written clears safe.
                pair = (2 * hp, 2 * hp + 1)
                b0 = 170 * slot
                for idx, h in enumerate(pair):
                    c0 = b0 + 65 * idx
                    for j in range(i + 1):
                        P, base = segs[h][j]
                        off = base + 128 * (i - j)
                        nc.tensor.matmul(
                            y2[:, c0:c0 + D + 1],
                            P[:, off:off + 128],
                            v_sb[j][:, h, :],
                            start=(j == 0), stop=(j == i),
                        )
                recip = small.tile([128, 2], fp32, tag="recip", name="recip")
                nc.vector.reciprocal(
                    recip[:],
                    y2[:, b0:b0 + 130].rearrange(
                        "p (g c) -> p g c", c=65)[:, :, D],
                )
                # both heads of the pair share one [128, 128] tile so a single
                # PE transpose handles the pair (head parity = column halves).
                # The last cycle normalizes on ACT (idle there) to keep the
                # DVE FIFO off the AV->normalize->AV latency loop.
                yn = small.tile([128, 2 * D], bf16, tag="yn", name="yn",
                                bufs=18)
                Copy = mybir.ActivationFunctionType.Copy
                for idx, h in enumerate(pair):
                    c0 = b0 + 65 * idx
                    if hp == NPAIR - 1:
                        nc.scalar.activation(yn[:, D * idx:D * idx + D],
                                             y2[:, c0:c0 + D], Copy,
                                             scale=recip[:, idx:idx + 1])
                    else:
                        nc.vector.tensor_scalar_mul(yn[:, D * idx:D * idx + D],
                                                    y2[:, c0:c0 + D],
                                                    recip[:, idx:idx + 1])
                yns.append((i, yn))

            def emit_yT(hp, trs):
                nc.vector.tensor_scalar_add(
                    yT_sb[hp][:],
                    trs[:],
                    bqkv_sb[:, 2 * KC + hp:2 * KC + hp + 1],
                )

            def emit_yT_slice(hp, trs, i):
                # per-step slice add (last cycle): lets proj kc=5 chunks for
                # tile i start as soon as transpose(i) lands
                sl = slice(128 * i, 128 * (i + 1))
                nc.vector.tensor_scalar_add(
                    yT_sb[hp][:, sl],
                    trs[:, sl],
                    bqkv_sb[:, 2 * KC + hp:2 * KC + hp + 1],
                )

            def new_segs():
                return {h: {} for h in range(H)}

            # ---- cold start: chunk-major qk(0) — all four halves advance
            # one contraction chunk at a time so PE tracks DMA arrivals ----
            segs = {0: new_segs()}
            qkA = ps_big.tile([128, 1024], fp32, tag="big", name="qkA")
            qkB = ps_big.tile([128, 1024], fp32, tag="big", name="qkB")
            for kc in range(KC):
                f, l = (kc == 0), (kc == KC - 1)
                wq = wqkv_sb[kc][:, 0:128]
                wk = wqkv_sb[kc][:, C:C + 128]
                for ps_, sl in ((qkA, slice(0, 512)), (qkB, slice(512, 1024))):
                    nc.tensor.matmul(ps_[:, 0:512], wq, xT_sb[kc][:, sl],
                                     start=f, stop=l)
                    nc.tensor.matmul(ps_[:, 512:1024], wk, xT_sb[kc][:, sl],
                                     start=f, stop=l)
            # add order matches S(0)-j0 consumption: k-t0 first (its 0:128
            # sliver gates the first S Ldweights), then q halves, k-t1 last
            for ps_, col, sl in (
                (qkA, KC, slice(0, 512)), (qkA, 0, slice(0, 512)),
                (qkB, 0, slice(512, 1024)), (qkB, KC, slice(512, 1024)),
            ):
                src = ps_[:, 0:512] if col == 0 else ps_[:, 512:1024]
                nc.vector.tensor_scalar_add(
                    qkT_sb[col][:, sl], src, bqkv_sb[:, col:col + 1])
            bps = ps_sm.tile([128, 512], fp32, tag="sm", name="bias_ps")
            nc.tensor.matmul(bps[:, 0:512], ones_sb[:], bproj_sb[:, 0:512],
                             start=True, stop=True)
            nc.vector.tensor_copy(bias_sb[:, 0:512], bps[:, 0:512])
            bps2 = ps_sm.tile([128, 512], fp32, tag="sm", name="bias_ps2")
            nc.tensor.matmul(bps2[:, 0:256], ones_sb[:], bproj_sb[:, 512:768],
                             start=True, stop=True)
            nc.vector.tensor_copy(bias_sb[:, 512:768], bps2[:, 0:256])
            for g in range(4):
                emit_S_group(0, segs[0], GROUPS[4 - g])
                emit_v(g)
            emit_S_j4_pair(0, segs[0])
            emit_v(4)
            emit_v(5)

            # ---- pipelined cycles ----
            # per cycle: 8 AV i-steps; qk(hp+1) halves at steps 0,1,3; S(hp+1)
            # groups j0-first at steps 2,4,5,6,7 (j0 consumed first next cycle).
            # Transposes are lag-0: yn(i-1) transposed at step i of the SAME
            # cycle, so yT(hp) lands at cycle end. The last cycle has no
            # qk/S(hp+1) work, so proj chunks kc<=4 for tiles 0-2 pre-fill
            # its PE stalls (big pool is otherwise unused there).
            qk_sched = {0: [("q", 0), ("k", 0)], 1: [("q", 1)], 3: [("k", 1)]}
            def emit_transpose_slice(trs, yns, sl):
                for i, yn in yns[sl]:
                    nc.tensor.transpose(
                        trs[:, 128 * i:128 * (i + 1)],
                        yn[:], ident_sb[:])

            def emit_proj_chunk(ps, tt, kc, start, stop):
                for a, b_ in ((0, 512), (512, 768)):
                    nc.tensor.matmul(
                        ps[:, a:b_],
                        yT_sb[kc][:, tt * 128:(tt + 1) * 128],
                        wproj_sb[kc][:, a:b_],
                        start=start, stop=stop,
                    )

            # Last-cycle proj schedule: tiles 2-7 fully accumulate (kc 0..5)
            # inside cycle 5 — tile tt's kc=5 chunk only needs the 128-col
            # slice of yT(5) produced by transpose(tt), which lands per-step
            # (sliced yT adds in the last cycle). Out-adds free big bufs so
            # three tiles are in flight at a time; tiles 0-1 run post-loop,
            # overlapping the drain.
            proj_ps = {}
            # step -> list of ("chunks", tt, kcs) | ("out", tt)
            c5_sched = {
                0: [("chunks", 2, (0, 1, 2))],
                1: [("chunks", 2, (3, 4)), ("chunks", 3, (0,))],
                2: [("chunks", 3, (1, 2, 3, 4))],
                3: [("chunks", 4, (0, 1, 2))],
                4: [("chunks", 4, (3, 4)), ("chunks", 2, (5,)), ("out", 2)],
                5: [("chunks", 5, (0, 1, 2)), ("chunks", 3, (5,)), ("out", 3)],
                6: [("chunks", 5, (3, 4)), ("chunks", 6, (0, 1, 2)),
                    ("chunks", 4, (5,)), ("out", 4)],
                7: [("chunks", 6, (3, 4)), ("chunks", 7, (0, 1, 2, 3, 4)),
                    ("chunks", 5, (5,)), ("out", 5)],
            }

            def emit_proj_out(tt, ps, piece=False):
                o = osb.tile([128, C], bf16, tag="o_sb", name="o_sb")
                q = nc.scalar if tt % 2 else nc.sync
                if not piece:
                    nc.vector.tensor_add(o[:], ps[:, 0:768], bias_sb[:])
                    q.dma_start(out[tt * 128:(tt + 1) * 128, :], o[:])
                else:
                    for a, b_ in ((0, 384), (384, 640), (640, 768)):
                        nc.vector.tensor_add(o[:, a:b_], ps[:, a:b_],
                                             bias_sb[:, a:b_])
                        q.dma_start(
                            out[tt * 128:(tt + 1) * 128, a:b_], o[:, a:b_])

            def emit_c5(action):
                kind, tt = action[0], action[1]
                if kind == "chunks":
                    if tt not in proj_ps:
                        proj_ps[tt] = ps_big.tile([128, 1024], fp32,
                                                  tag="big", name="o_ps")
                    for kc in action[2]:
                        emit_proj_chunk(proj_ps[tt], tt, kc,
                                        start=(kc == 0), stop=(kc == 5))
                else:
                    emit_proj_out(tt, proj_ps.pop(tt))

            y2_tile = ps_sm.tile([128, 512], fp32, tag="sm", name="y2")
            for hp in range(NPAIR):
                nxt = hp + 1 < NPAIR
                if nxt:
                    segs[hp + 1] = new_segs()
                yns = []
                trs = ps_tr.tile([128, 1024], bf16, tag="tr", name="tr")
                for i in range(TT):
                    emit_AV_half(hp, segs[hp], yns, i, y2_tile, (hp * TT + i) % 3)
                    if hp == 0 and i in (3, 5):
                        emit_v(6 if i == 3 else 7)
                    if i > 0:
                        emit_transpose_slice(trs, yns, slice(i - 1, i))
                        if not nxt:
                            emit_yT_slice(hp, trs, i - 1)
                    if nxt:
                        for args in qk_sched.get(i, []):
                            emit_qk_half(hp + 1, *args)
                        gidx = {2: 4, 4: 3, 5: 2, 6: 1}.get(i)
                        if gidx is not None:
                            emit_S_group(hp + 1, segs[hp + 1], GROUPS[gidx])
                        elif i == 7:
                            emit_S_j4_pair(hp + 1, segs[hp + 1])
                    else:
                        for action in c5_sched.get(i, []):
                            emit_c5(action)
                emit_transpose_slice(trs, yns, slice(TT - 1, TT))
                if nxt:
                    emit_yT(hp, trs)
                else:
                    emit_yT_slice(hp, trs, TT - 1)
                segs.pop(hp)

            # ---- drain: remaining kc5 chunks, then tiles 0-1 in full ----
            for action in (("chunks", 6, (5,)), ("out", 6),
                           ("chunks", 7, (5,)), ("out", 7)):
                emit_c5(action)
            for tt in (0, 1):
                ps = ps_big.tile([128, 1024], fp32, tag="big", name="o_ps")
                for kc in range(KC):
                    emit_proj_chunk(ps, tt, kc, start=(kc == 0),
                                    stop=(kc == KC - 1))
                emit_proj_out(tt, ps, piece=(tt == 1))

    nc.compile()
    return nc


def _prep_inputs(x, w_qkv, b_qkv, w_proj, b_proj):
    wqkv_bf = np.ascontiguousarray(w_qkv.astype(_BF16))
    wproj_bf = np.ascontiguousarray(w_proj.astype(_BF16))
    bqkv_pc = np.ascontiguousarray(b_qkv.astype(np.float32).reshape(C3 // 128, 128).T)
    bproj_bf = np.ascontiguousarray(b_proj.astype(_BF16).reshape(1, C))
    in_maps = []
    for b in range(B):
        xTb = np.ascontiguousarray(x[b].astype(_BF16).T)
        in_maps.append({
            "xT": xTb,
            "wqkv": wqkv_bf,
            "wproj": wproj_bf,
            "bqkv": bqkv_pc,
            "bproj": bproj_bf,
        })
    return in_maps


def _run(inputs, trace=False):
    from concourse.bass_utils import run_bass_kernel_spmd

    if "nc" not in _compiled:
        _compiled["nc"] = _build()
    nc = _compiled["nc"]
    in_maps = _prep_inputs(inputs["x"], inputs["w_qkv"], inputs["b_qkv"],
                           inputs["w_proj"], inputs["b_proj"])
    res = run_bass_kernel_spmd(nc, in_maps, list(range(B)), trace=trace)
    outs = np.stack([np.asarray(res.results[b]["out"]) for b in range(B)])
    return outs.astype(np.float32), res


def kernel(x, w_qkv, b_qkv, w_proj, b_proj):
    outs, _ = _run(dict(x=x, w_qkv=w_qkv, b_qkv=b_qkv,
                        w_proj=w_proj, b_proj=b_proj))
    return outs



# revision 38
# speedup vs baseline: 1.0224x; 1.0186x over previous
"""Causal self-attention (B=8, T=1024, C=768, H=12, D=64) on 8 TRN2 NeuronCores.

Sharding: data-parallel over batch — core b handles batch element b. No
collectives. Host pre-transposes x to x^T[b] and pre-casts operands to bf16;
all matmuls run bf16 with fp32 PSUM accumulation.

Per-core algorithm:
  v = x Wv in [t, c] layout (x^T stationary); v bias folded into the y^T
  stage (exact: softmax rows sum to 1). q^T,k^T = (Wqkv^T x^T + b) in
  [c3, t] layout (weights stationary). Per head h, key-block j (128 keys):
  S^T = K_j Q^T in PSUM [keys, q] (causal: only q >= 128j columns; blocks
  pack into [128,1024] PSUM tiles as {j0},{j1,j7},{j2,j6},{j3,j5},{j4} so
  one ACT exp covers each tile, scale=1/8), triangular mask-multiply on
  diagonal 128x128 blocks. Per q-tile i: y'[q,65] = sum_j P_j^T.T @ [V_j|1]
  accumulated in PSUM (two heads x two i-steps share one PSUM bank); col 64
  is the softmax denominator. Normalize via per-partition reciprocal+scale,
  PE-transpose into a per-pair [128, 1024] bf16 PSUM strip (head parity in
  partition halves), one DVE pass per pair adds the v-bias and lands y^T in
  SBUF. out[t, c] = y^T.T @ Wproj + b_proj (bias via K=1 ones matmul).

Emission is software-pipelined (static per-engine order => head-of-line
blocking): cycle hp interleaves AV(hp) i-steps with qk(hp+1) half-chunks
and S(hp+1) groups so PE fills ACT-paced exp stalls; v tiles fill the S(0)
cold start.

PSUM budget (8 banks): big [128,1024]fp32 x2 (4, shared v/qk/S/o) +
y' [128,512]fp32 x2 (2, two heads x two i-steps packed) +
tr [128,1024]bf16 x2 (2).
"""

import numpy as np
import ml_dtypes

B, T, C = 8, 1024, 768
H, D = 12, 64
C3 = 3 * C
KC = C // 128          # 6 contraction chunks over c_in
TT = T // 128          # 8 t-tiles of 128
NPAIR = H // 2

BIG_BUFS = 3
SM_BUFS = 1
TR_BUFS = 1
PP_BUFS = 20           # 10 P segs live per pair (5 groups x 2 heads)

_BF16 = ml_dtypes.bfloat16

_compiled = {}


def _build():
    from concourse import bacc, mybir
    import concourse.tile as tile
    from concourse.masks import make_identity, make_upper_triangular

    fp32 = mybir.dt.float32
    bf16 = mybir.dt.bfloat16

    nc = bacc.Bacc("TRN2", target_bir_lowering=False, debug=False,
                   enable_asserts=True, num_devices=B)

    xT = nc.dram_tensor("xT", [C, T], bf16, kind="ExternalInput")
    wqkv = nc.dram_tensor("wqkv", [C, C3], bf16, kind="ExternalInput")
    wproj = nc.dram_tensor("wproj", [C, C], bf16, kind="ExternalInput")
    # b_qkv rearranged host-side to [128, 18]: col j holds b_qkv[128j:128j+128]
    bqkv = nc.dram_tensor("bqkv", [128, C3 // 128], fp32, kind="ExternalInput")
    bproj = nc.dram_tensor("bproj", [1, C], bf16, kind="ExternalInput")
    out = nc.dram_tensor("out", [T, C], bf16, kind="ExternalOutput")

    Exp = mybir.ActivationFunctionType.Exp
    # S-block packing: groups of (j, base column) sharing one [128,1024]
    # PSUM tile => one exp per tile. Bases keep each block inside the tile.
    GROUPS = [((4, 0),), ((3, 0), (5, 640)), ((2, 0), (6, 768)),
              ((1, 0), (7, 896)), ((0, 0),)]

    with tile.TileContext(nc) as tc:
        with (
            tc.tile_pool(name="const", bufs=1) as const,
            tc.tile_pool(name="pP", bufs=PP_BUFS) as pP,
            tc.tile_pool(name="small", bufs=6) as small,
            tc.tile_pool(name="osb", bufs=4) as osb,
            tc.tile_pool(name="ps_big", bufs=BIG_BUFS, space="PSUM") as ps_big,
            tc.tile_pool(name="ps_sm", bufs=SM_BUFS, space="PSUM") as ps_sm,
            tc.tile_pool(name="ps_tr", bufs=TR_BUFS, space="PSUM") as ps_tr,
        ):
            # ---- persistent SBUF loads ----
            # Merged [128, KC, *] tiles allow few, large, >=512B-row DMAs.
            # Critical path: per-chunk q/k pair-0/1 slivers (SP queue)
            # interleaved with per-chunk full x^T (ACT queue) feed a
            # chunk-major qk(0); everything else streams behind them.
            xT_all = const.tile([128, KC, T], bf16, tag="xT", name="xT_all")
            wqkv_all = const.tile([128, KC, C3], bf16, tag="wqkv",
                                  name="wqkv_all")
            wproj_all = const.tile([128, KC, C], bf16, tag="wproj",
                                   name="wproj_all")
            xT_sb = [xT_all[:, kc] for kc in range(KC)]
            wqkv_sb = [wqkv_all[:, kc] for kc in range(KC)]
            wproj_sb = [wproj_all[:, kc] for kc in range(KC)]
            wqkv_pac = wqkv.rearrange("(a p) c -> p a c", p=128)
            bqkv_sb = const.tile([128, C3 // 128], fp32, tag="bqkv", name="bqkv")
            bproj_sb = const.tile([1, C], bf16, tag="bproj", name="bproj")
            nc.gpsimd.dma_start(bqkv_sb[:], bqkv[:, :])
            nc.gpsimd.dma_start(bproj_sb[:], bproj[:, :])
            for kc in range(KC):
                rows = slice(kc * 128, (kc + 1) * 128)
                nc.sync.dma_start(
                    wqkv_all[:, kc, 0:2 * C].rearrange(
                        "p (a b) -> p a b", a=2)[:, :, 0:256],
                    wqkv[rows, 0:2 * C].rearrange(
                        "p (a b) -> p a b", a=2)[:, :, 0:256],
                )
                nc.scalar.dma_start(xT_all[:, kc], xT[rows, :])
            # Bulk loads ride the same two HWDGE queues BEHIND the critical
            # stream (SWDGE would jump the shared DMA device): v columns +
            # q-rest on SP, k-rest + wproj on ACT.
            nc.sync.dma_start(wqkv_all[:, :, 2 * C:2 * C + 512],
                              wqkv_pac[:, :, 2 * C:2 * C + 512])
            nc.sync.dma_start(wqkv_all[:, :, 2 * C + 512:C3],
                              wqkv_pac[:, :, 2 * C + 512:C3])
            nc.sync.dma_start(wqkv_all[:, :, 256:768],
                              wqkv_pac[:, :, 256:768])
            nc.scalar.dma_start(wqkv_all[:, :, C + 256:C + 768],
                                wqkv_pac[:, :, C + 256:C + 768])
            nc.scalar.dma_start(wproj_all[:],
                                wproj.rearrange("(a p) c -> p a c", p=128))
            ones_sb = const.tile([1, 128], bf16, tag="ones", name="ones")
            nc.vector.memset(ones_sb[:], 1.0)
            # b_proj broadcast to all 128 partitions (PE trick, done once in
            # the prologue); proj adds it via DVE instead of per-tile matmuls
            bias_sb = const.tile([128, C], bf16, tag="bias", name="bias")
            ident_sb = const.tile([128, 128], bf16, tag="ident", name="ident")
            make_identity(nc, ident_sb[:])
            # keep columns m >= l (query >= key) on the diagonal block
            trimask_sb = const.tile([128, 128], bf16, tag="trimask", name="trimask")
            make_upper_triangular(nc, trimask_sb[:], val=1.0, diag=True)

            qkT_sb = [const.tile([128, T], bf16, tag=f"qkT{c3}", name=f"qkT{c3}")
                      for c3 in range(2 * KC)]
            # v packed [t, 12 heads x (64 + ones col)]
            v_sb = [const.tile([128, H, D + 1], bf16, tag=f"v{tt}", name=f"v{tt}")
                    for tt in range(TT)]
            yT_sb = [const.tile([128, T], bf16, tag=f"yT{kc}", name=f"yT{kc}")
                     for kc in range(KC)]

            def emit_v(tt):
                ps = ps_big.tile([128, 1024], fp32, tag="big", name="v_ps")
                for kc in range(KC):
                    nc.tensor.matmul(
                        ps[:, 0:512],
                        xT_sb[kc][:, tt * 128:(tt + 1) * 128],
                        wqkv_sb[kc][:, 2 * C:2 * C + 512],
                        start=(kc == 0), stop=(kc == KC - 1),
                    )
                    nc.tensor.matmul(
                        ps[:, 512:768],
                        xT_sb[kc][:, tt * 128:(tt + 1) * 128],
                        wqkv_sb[kc][:, 2 * C + 512:3 * C],
                        start=(kc == 0), stop=(kc == KC - 1),
                    )
                vv = v_sb[tt]
                nc.vector.tensor_copy(
                    vv[:, :, 0:D],
                    ps[:, 0:768].rearrange("p (h d) -> p h d", d=D),
                )
                nc.vector.memset(vv[:, :, D:D + 1], 1.0)

            def emit_qk_half(hp, which, tchunk):
                c3 = hp if which == "q" else KC + hp
                ps = ps_big.tile([128, 1024], fp32, tag="big", name="qk_ps")
                sl = slice(tchunk * 512, (tchunk + 1) * 512)
                for kc in range(KC):
                    nc.tensor.matmul(
                        ps[:, sl],
                        wqkv_sb[kc][:, c3 * 128:(c3 + 1) * 128],
                        xT_sb[kc][:, sl],
                        start=(kc == 0), stop=(kc == KC - 1),
                    )
                nc.vector.tensor_scalar_add(
                    qkT_sb[c3][:, sl], ps[:, sl], bqkv_sb[:, c3:c3 + 1],
                )

            def emit_S_group(hp, segs, grp):
                qT = qkT_sb[hp]
                kT = qkT_sb[KC + hp]
                for h in (2 * hp, 2 * hp + 1):
                    poff = 64 * (h % 2)
                    S = ps_big.tile([128, 1024], fp32, tag="big", name="S")
                    span = 0
                    for j, base in grp:
                        qs = 128 * j
                        w = T - qs
                        span = base + w
                        first = base + min(512 - base % 512, w) if base < 512 \
                            else base + w
                        for a, b_ in ((base, first), (first, base + w)):
                            if b_ <= a:
                                continue
                            nc.tensor.matmul(
                                S[:, a:b_],
                                kT[poff:poff + 64, qs:qs + 128],
                                qT[poff:poff + 64, qs + (a - base):qs + (b_ - base)],
                                start=True, stop=True,
                            )
                    P = pP.tile([128, 1024], bf16, tag="P", name="P")
                    nc.scalar.activation(P[:, 0:span], S[:, 0:span], Exp,
                                         scale=0.125)
                    for j, base in grp:
                        nc.vector.tensor_mul(P[:, base:base + 128],
                                             P[:, base:base + 128],
                                             trimask_sb[:])
                        segs[h][j] = (P, base)

            def emit_S_j4_pair(hp, segs):
                # both heads' j4 block (512 cols each) share one PSUM tile
                # and one exp: halves the ACT op overhead for this group
                qT = qkT_sb[hp]
                kT = qkT_sb[KC + hp]
                S = ps_big.tile([128, 1024], fp32, tag="big", name="S")
                for idx, h in enumerate((2 * hp, 2 * hp + 1)):
                    poff = 64 * (h % 2)
                    nc.tensor.matmul(
                        S[:, 512 * idx:512 * idx + 512],
                        kT[poff:poff + 64, 512:640],
                        qT[poff:poff + 64, 512:1024],
                        start=True, stop=True,
                    )
                P = pP.tile([128, 1024], bf16, tag="P", name="P")
                nc.scalar.activation(P[:], S[:], Exp, scale=0.125)
                for idx, h in enumerate((2 * hp, 2 * hp + 1)):
                    base = 512 * idx
                    nc.vector.tensor_mul(P[:, base:base + 128],
                                         P[:, base:base + 128],
                                         trimask_sb[:])
                    segs[h][4] = (P, base)

            def emit_AV_half(hp, segs, yns, i, y2, slot):
                # y2 is one persistent [128, 512] PSUM tile; i-steps rotate
                # through three 170-col slots (2 heads x 65 used). Subtile dep
                # tracking gives each new accumulation 3 steps of slack over
                # the previous slot's readers; PE program order makes the
                # in-bank has_written clears safe.
                pair = (2 * hp, 2 * hp + 1)
                b0 = 170 * slot
                for idx, h in enumerate(pair):
                    c0 = b0 + 65 * idx
                    for j in range(i + 1):
                        P, base = segs[h][j]
                        off = base + 128 * (i - j)
                        nc.tensor.matmul(
                            y2[:, c0:c0 + D + 1],
                            P[:, off:off + 128],
                            v_sb[j][:, h, :],
                            start=(j == 0), stop=(j == i),
                        )
                recip = small.tile([128, 2], fp32, tag="recip", name="recip")
                nc.vector.reciprocal(
                    recip[:],
                    y2[:, b0:b0 + 130].rearrange(
                        "p (g c) -> p g c", c=65)[:, :, D],
                )
                # both heads of the pair share one [128, 128] tile so a single
                # PE transpose handles the pair (head parity = column halves).
                # The last cycle normalizes on ACT (idle there) to keep the
                # DVE FIFO off the AV->normalize->AV latency loop.
                yn = small.tile([128, 2 * D], bf16, tag="yn", name="yn",
                                bufs=18)
                Copy = mybir.ActivationFunctionType.Copy
                for idx, h in enumerate(pair):
                    c0 = b0 + 65 * idx
                    if hp == NPAIR - 1:
                        nc.scalar.activation(yn[:, D * idx:D * idx + D],
                                             y2[:, c0:c0 + D], Copy,
                                             scale=recip[:, idx:idx + 1])
                    else:
                        nc.vector.tensor_scalar_mul(yn[:, D * idx:D * idx + D],
                                                    y2[:, c0:c0 + D],
                                                    recip[:, idx:idx + 1])
                yns.append((i, yn))

            def emit_yT(hp, trs):
                nc.vector.tensor_scalar_add(
                    yT_sb[hp][:],
                    trs[:],
                    bqkv_sb[:, 2 * KC + hp:2 * KC + hp + 1],
                )

            def emit_yT_slice(hp, trs, i):
                # per-step slice add (last cycle): lets proj kc=5 chunks for
                # tile i start as soon as transpose(i) lands
                sl = slice(128 * i, 128 * (i + 1))
                nc.vector.tensor_scalar_add(
                    yT_sb[hp][:, sl],
                    trs[:, sl],
                    bqkv_sb[:, 2 * KC + hp:2 * KC + hp + 1],
                )

            def new_segs():
                return {h: {} for h in range(H)}

            # ---- cold start: chunk-major qk(0) — all four halves advance
            # one contraction chunk at a time so PE tracks DMA arrivals ----
            segs = {0: new_segs()}
            qkA = ps_big.tile([128, 1024], fp32, tag="big", name="qkA")
            qkB = ps_big.tile([128, 1024], fp32, tag="big", name="qkB")
            for kc in range(KC):
                f, l = (kc == 0), (kc == KC - 1)
                wq = wqkv_sb[kc][:, 0:128]
                wk = wqkv_sb[kc][:, C:C + 128]
                for ps_, sl in ((qkA, slice(0, 512)), (qkB, slice(512, 1024))):
                    nc.tensor.matmul(ps_[:, 0:512], wq, xT_sb[kc][:, sl],
                                     start=f, stop=l)
                    nc.tensor.matmul(ps_[:, 512:1024], wk, xT_sb[kc][:, sl],
                                     start=f, stop=l)
            # add order matches S(0)-j0 consumption: k-t0 first (its 0:128
            # sliver gates the first S Ldweights), then q halves, k-t1 last
            for ps_, col, sl in (
                (qkA, KC, slice(0, 512)), (qkA, 0, slice(0, 512)),
                (qkB, 0, slice(512, 1024)), (qkB, KC, slice(512, 1024)),
            ):
                src = ps_[:, 0:512] if col == 0 else ps_[:, 512:1024]
                nc.vector.tensor_scalar_add(
                    qkT_sb[col][:, sl], src, bqkv_sb[:, col:col + 1])
            bps = ps_sm.tile([128, 512], fp32, tag="sm", name="bias_ps")
            nc.tensor.matmul(bps[:, 0:512], ones_sb[:], bproj_sb[:, 0:512],
                             start=True, stop=True)
            nc.vector.tensor_copy(bias_sb[:, 0:512], bps[:, 0:512])
            bps2 = ps_sm.tile([128, 512], fp32, tag="sm", name="bias_ps2")
            nc.tensor.matmul(bps2[:, 0:256], ones_sb[:], bproj_sb[:, 512:768],
                             start=True, stop=True)
            nc.vector.tensor_copy(bias_sb[:, 512:768], bps2[:, 0:256])
            for g in range(4):
                emit_S_group(0, segs[0], GROUPS[4 - g])
                emit_v(g)
            emit_S_j4_pair(0, segs[0])
            emit_v(4)
            emit_v(5)

            # ---- pipelined cycles ----
            # per cycle: 8 AV i-steps; qk(hp+1) halves at steps 0,1,3; S(hp+1)
            # groups j0-first at steps 2,4,5,6,7 (j0 consumed first next cycle).
            # Transposes are lag-0: yn(i-1) transposed at step i of the SAME
            # cycle, so yT(hp) lands at cycle end. The last cycle has no
            # qk/S(hp+1) work, so proj chunks kc<=4 for tiles 0-2 pre-fill
            # its PE stalls (big pool is otherwise unused there).
            qk_sched = {0: [("q", 0), ("k", 0)], 1: [("q", 1)], 3: [("k", 1)]}
            def emit_transpose_slice(trs, yns, sl):
                for i, yn in yns[sl]:
                    nc.tensor.transpose(
                        trs[:, 128 * i:128 * (i + 1)],
                        yn[:], ident_sb[:])

            def emit_proj_chunk(ps, tt, kc, start, stop):
                for a, b_ in ((0, 512), (512, 768)):
                    nc.tensor.matmul(
                        ps[:, a:b_],
                        yT_sb[kc][:, tt * 128:(tt + 1) * 128],
                        wproj_sb[kc][:, a:b_],
                        start=start, stop=stop,
                    )

            # Last-cycle proj schedule: tiles 2-7 fully accumulate (kc 0..5)
            # inside cycle 5 — tile tt's kc=5 chunk only needs the 128-col
            # slice of yT(5) produced by transpose(tt), which lands per-step
            # (sliced yT adds in the last cycle). Out-adds free big bufs so
            # three tiles are in flight at a time; tiles 0-1 run post-loop,
            # overlapping the drain.
            proj_ps = {}
            # step -> list of ("chunks", tt, kcs) | ("out", tt)
            c5_sched = {
                0: [("chunks", 2, (0, 1, 2))],
                1: [("chunks", 2, (3, 4)), ("chunks", 3, (0,))],
                2: [("chunks", 3, (1, 2, 3, 4))],
                3: [("chunks", 4, (0, 1, 2))],
                4: [("chunks", 4, (3, 4)), ("chunks", 2, (5,)), ("out", 2)],
                5: [("chunks", 5, (0, 1, 2)), ("chunks", 3, (5,)), ("out", 3)],
                6: [("chunks", 5, (3, 4)), ("chunks", 6, (0, 1, 2)),
                    ("chunks", 4, (5,)), ("out", 4)],
                7: [("chunks", 6, (3, 4)), ("chunks", 7, (0, 1, 2, 3, 4)),
                    ("chunks", 5, (5,)), ("out", 5)],
            }

            def emit_proj_out(tt, ps, piece=False):
                o = osb.tile([128, C], bf16, tag="o_sb", name="o_sb")
                q = nc.scalar if tt % 2 else nc.sync
                if not piece:
                    nc.vector.tensor_add(o[:], ps[:, 0:768], bias_sb[:])
                    q.dma_start(out[tt * 128:(tt + 1) * 128, :], o[:])
                else:
                    for a, b_ in ((0, 384), (384, 640), (640, 768)):
                        nc.vector.tensor_add(o[:, a:b_], ps[:, a:b_],
                                             bias_sb[:, a:b_])
                        q.dma_start(
                            out[tt * 128:(tt + 1) * 128, a:b_], o[:, a:b_])

            def emit_c5(action):
                kind, tt = action[0], action[1]
                if kind == "chunks":
                    if tt not in proj_ps:
                        proj_ps[tt] = ps_big.tile([128, 1024], fp32,
                                                  tag="big", name="o_ps")
                    for kc in action[2]:
                        emit_proj_chunk(proj_ps[tt], tt, kc,
                                        start=(kc == 0), stop=(kc == 5))
                else:
                    emit_proj_out(tt, proj_ps.pop(tt))

            y2_tile = ps_sm.tile([128, 512], fp32, tag="sm", name="y2")
            for hp in range(NPAIR):
                nxt = hp + 1 < NPAIR
                if nxt:
                    segs[hp + 1] = new_segs()
                yns = []
                trs = ps_tr.tile([128, 1024], bf16, tag="tr", name="tr")
                for i in range(TT):
                    emit_AV_half(hp, segs[hp], yns, i, y2_tile, (hp * TT + i) % 3)
                    if hp == 0 and i in (3, 5):
                        emit_v(6 if i == 3 else 7)
                    if i > 0:
                        emit_transpose_slice(trs, yns, slice(i - 1, i))
                        if not nxt:
                            emit_yT_slice(hp, trs, i - 1)
                    if nxt:
                        for args in qk_sched.get(i, []):
                            emit_qk_half(hp + 1, *args)
                        gidx = {2: 4, 4: 3, 5: 2, 6: 1}.get(i)
                        if gidx is not None:
                            emit_S_group(hp + 1, segs[hp + 1], GROUPS[gidx])
                        elif i == 7:
                            emit_S_j4_pair(hp + 1, segs[hp + 1])
                    else:
                        for action in c5_sched.get(i, []):
                            emit_c5(action)
                emit_transpose_slice(trs, yns, slice(TT - 1, TT))
                if nxt:
                    emit_yT(hp, trs)
                else:
                    emit_yT_slice(hp, trs, TT - 1)
                segs.pop(hp)

            # ---- drain: remaining kc5 chunks, then tiles 0-1 in full ----
            for action in (("chunks", 6, (5,)), ("out", 6),
                           ("chunks", 7, (5,)), ("out", 7)):
                emit_c5(action)
            for tt in (0, 1):
                ps = ps_big.tile([128, 1024], fp32, tag="big", name="o_ps")
                for kc in range(KC):
                    emit_proj_chunk(ps, tt, kc, start=(kc == 0),
                                    stop=(kc == KC - 1))
                emit_proj_out(tt, ps, piece=(tt == 1))

    nc.compile()
    return nc


def _prep_inputs(x, w_qkv, b_qkv, w_proj, b_proj):
    wqkv_bf = np.ascontiguousarray(w_qkv.astype(_BF16))
    wproj_bf = np.ascontiguousarray(w_proj.astype(_BF16))
    bqkv_pc = np.ascontiguousarray(b_qkv.astype(np.float32).reshape(C3 // 128, 128).T)
    bproj_bf = np.ascontiguousarray(b_proj.astype(_BF16).reshape(1, C))
    in_maps = []
    for b in range(B):
        xTb = np.ascontiguousarray(x[b].astype(_BF16).T)
        in_maps.append({
            "xT": xTb,
            "wqkv": wqkv_bf,
            "wproj": wproj_bf,
            "bqkv": bqkv_pc,
            "bproj": bproj_bf,
        })
    return in_maps


def _run(inputs, trace=False):
    from concourse.bass_utils import run_bass_kernel_spmd

    if "nc" not in _compiled:
        _compiled["nc"] = _build()
    nc = _compiled["nc"]
    in_maps = _prep_inputs(inputs["x"], inputs["w_qkv"], inputs["b_qkv"],
                           inputs["w_proj"], inputs["b_proj"])
    res = run_bass_kernel_spmd(nc, in_maps, list(range(B)), trace=trace)
    outs = np.stack([np.asarray(res.results[b]["out"]) for b in range(B)])
    return outs.astype(np.float32), res


def kernel(x, w_qkv, b_qkv, w_proj, b_proj):
    outs, _ = _run(dict(x=x, w_qkv=w_qkv, b_qkv=b_qkv,
                        w_proj=w_proj, b_proj=b_proj))
    return outs

